# revision 35
# baseline (speedup 1.0000x reference)
"""Trainium2 Bass kernel for nn_DiTBlock (B=4,N=1024,C=1024,H=16).

8-way SPMD: core i handles batch i//2, row-half i%2 (512 query rows).

- All big GEMMs run as fp8e4 DoubleRow matmuls (2 K-tiles/instruction,
  0.5 cyc/row = 4x bf16); scores and attn@V stay fp8 (attn@V DoubleRow
  over key-tile pairs). MLP accuracy is recovered with hi+lo fp8
  residual splits (fc1: 3 passes incl. an x-hat lo pass; fc2: 2 passes),
  keeping rel err ~1.2e-2 vs the 2e-2 gate.
- Keys are host-packed to the unmasked subset padded to NKV=640 (the
  mask is a kernel input, so packing is host-side work), shrinking
  exp/scores/attn@V/kT/V by ~40%.
- Softmax in S^T layout: mask rides the exp bias per partition, the
  denominator comes from an all-ones fp8 DoubleRow matmul, normalize =
  DVE reciprocal + multiply (no ISA divide).
- q/k/fc1 weights are stored x32 in fp8 to avoid subnormals; the exp /
  gelu activation scale compensates.
- Scheduling: ca k/v GEMMs run first (fill LN1 window), attn@V of group
  g-1 interleaves into group g's scores so PE works during the exp
  chain, per-engine queues balanced (exp/gelu/evac on Act, LN + divides
  on DVE, SBUF-only work on Pool), fc2 hi-weights prefetched during ca
  attention.
"""
import numpy as np
from contextlib import ExitStack

import concourse.bass as bass
import concourse.bacc as bacc
import concourse.mybir as mybir
import concourse.tile as tile
from concourse.bass_utils import run_bass_kernel_spmd
from concourse.masks import make_identity

F32 = mybir.dt.float32
F32R = mybir.dt.float32r
BF16 = mybir.dt.bfloat16
FP8 = mybir.dt.float8e4
AF = mybir.ActivationFunctionType
ALU = mybir.AluOpType
DR = mybir.MatmulPerfMode.DoubleRow

B, N, C, H, D = 4, 1024, 1024, 16, 64
HID = 4 * C
R = 512            # own query rows per core
NKV = 640          # packed+padded key count (binomial(1024,.5) <= 640 at 8 sigma)
KYV = NKV // 128   # 5 key tiles
KT = C // 128      # 8
EPS = 1e-6
WS = 32.0          # fp8 weight prescale for q/k/fc1
SCL = 0.125 / (WS * WS)   # exp scale compensating q~ = 32q, k~ = 32k

_cache = {}


def build_program(debug=False):
    nc = bacc.Bacc(None, target_bir_lowering=False)
    dbg = {}

    # ---------------- DRAM handles ----------------
    x_own = nc.dram_tensor("x_own", [R, C], F32, kind="ExternalInput")
    x_kv = nc.dram_tensor("x_kv", [NKV, C], BF16, kind="ExternalInput")
    cT_d = nc.dram_tensor("cT", [C, NKV], FP8, kind="ExternalInput")
    mask_sa = nc.dram_tensor("mask_sa", [128, KYV], F32, kind="ExternalInput")
    mask_ca = nc.dram_tensor("mask_ca", [128, KYV], F32, kind="ExternalInput")
    # weights (fp8; q/k cols head-permuted and x32, v cols plain)
    w_qkv = nc.dram_tensor("w_qkv", [C, 3 * C], FP8, kind="ExternalInput")
    w_proj = nc.dram_tensor("w_proj", [C, C], FP8, kind="ExternalInput")
    w_caq = nc.dram_tensor("w_caq", [C, C], FP8, kind="ExternalInput")
    w_cak = nc.dram_tensor("w_cak", [C, C], FP8, kind="ExternalInput")
    w_cav = nc.dram_tensor("w_cav", [C, C], FP8, kind="ExternalInput")
    w_caproj = nc.dram_tensor("w_caproj", [C, C], FP8, kind="ExternalInput")
    w_fc1h = nc.dram_tensor("w_fc1h", [C, HID], FP8, kind="ExternalInput")
    w_fc1l = nc.dram_tensor("w_fc1l", [C, HID], FP8, kind="ExternalInput")
    w_fc2h = nc.dram_tensor("w_fc2h", [HID, C], FP8, kind="ExternalInput")
    w_fc2l = nc.dram_tensor("w_fc2l", [HID, C], FP8, kind="ExternalInput")
    # biases: transposed column layouts [128, n_tiles] f32 (q/k x32+perm)
    b_qT = nc.dram_tensor("b_qT", [128, 8], F32, kind="ExternalInput")
    b_kT = nc.dram_tensor("b_kT", [128, 8], F32, kind="ExternalInput")
    b_caqT = nc.dram_tensor("b_caqT", [128, 8], F32, kind="ExternalInput")
    b_cakT = nc.dram_tensor("b_cakT", [128, 8], F32, kind="ExternalInput")
    b_fc1T = nc.dram_tensor("b_fc1T", [128, 32], F32, kind="ExternalInput")
    b_fc2T = nc.dram_tensor("b_fc2T", [128, 8], F32, kind="ExternalInput")
    # replicated-row biases [1, C] f32 (broadcast-DMA'd)
    b_vr = nc.dram_tensor("b_vr", [1, C], F32, kind="ExternalInput")
    b_cavr = nc.dram_tensor("b_cavr", [1, C], F32, kind="ExternalInput")
    b_projr = nc.dram_tensor("b_projr", [1, C], F32, kind="ExternalInput")
    b_caprojr = nc.dram_tensor("b_caprojr", [1, C], F32, kind="ExternalInput")
    yT = nc.dram_tensor("yT", [C, R], F32, kind="ExternalOutput")

    with tile.TileContext(nc) as tc, ExitStack() as ctx:
        misc = ctx.enter_context(tc.tile_pool(name="misc", bufs=1))
        pxp = ctx.enter_context(tc.tile_pool(name="pxp", bufs=1))
        lnrmp = ctx.enter_context(tc.tile_pool(name="lnrmp", bufs=5))
        statp = ctx.enter_context(tc.tile_pool(name="statp", bufs=4))
        finp = ctx.enter_context(tc.tile_pool(name="finp", bufs=2))
        wcolp = ctx.enter_context(tc.tile_pool(name="wcolp", bufs=4))
        psmm = ctx.enter_context(tc.tile_pool(name="psmm", bufs=2, space="PSUM"))

        x_sb = pxp.tile([128, 4, C], F32, tag="x")
        for rt in range(4):
            nc.gpsimd.dma_start(out=x_sb[:, rt, :],
                                in_=x_own[rt * 128:(rt + 1) * 128, :])

        ident32 = misc.tile([128, 128], F32)
        make_identity(nc, ident32)
        ident = misc.tile([128, 128], F32R)
        nc.vector.tensor_copy(ident, ident32)
        onesV = misc.tile([128, 2, 64], FP8)
        nc.vector.memset(onesV, 1.0)
        ones_r = misc.tile([1, 128], BF16)
        nc.gpsimd.memset(ones_r, 1.0)
        eps_b = misc.tile([128, 1], F32)
        nc.gpsimd.memset(eps_b, EPS)
        msk_sa = misc.tile([128, KYV], F32)
        nc.gpsimd.dma_start(out=msk_sa, in_=mask_sa[:, :])
        msk_ca = misc.tile([128, KYV], F32)
        nc.gpsimd.dma_start(out=msk_ca, in_=mask_ca[:, :])
        bqT = misc.tile([128, 8], F32)
        nc.gpsimd.dma_start(out=bqT, in_=b_qT[:, :])
        bkT = misc.tile([128, 8], F32)
        nc.gpsimd.dma_start(out=bkT, in_=b_kT[:, :])
        bcaqT = misc.tile([128, 8], F32)
        nc.gpsimd.dma_start(out=bcaqT, in_=b_caqT[:, :])
        bcakT = misc.tile([128, 8], F32)
        nc.gpsimd.dma_start(out=bcakT, in_=b_cakT[:, :])
        bfc1T = misc.tile([128, 32], F32)
        nc.gpsimd.dma_start(out=bfc1T, in_=b_fc1T[:, :])
        bfc2T = misc.tile([128, 8], F32)
        nc.gpsimd.dma_start(out=bfc2T, in_=b_fc2T[:, :])

        def bcast_load(dst, src_handle):
            s = src_handle[0:1, :]
            ap = bass.AP(tensor=s.tensor, offset=s.offset, ap=[[0, 128], [1, C]])
            nc.gpsimd.dma_start(out=dst, in_=ap)

        bvrow = misc.tile([1, C], BF16)
        nc.gpsimd.dma_start(out=bvrow, in_=b_vr[0:1, :])
        bcavrow = misc.tile([1, C], BF16)
        nc.gpsimd.dma_start(out=bcavrow, in_=b_cavr[0:1, :])
        brep_proj = misc.tile([128, C], F32)
        bcast_load(brep_proj, b_projr)
        brep_caproj = misc.tile([128, C], F32)
        bcast_load(brep_caproj, b_caprojr)

        # ---------------- LN helpers (baseline bn_stats + rsqrt bit-trick) --
        def ln_stats(src_ap):
            st = statp.tile([128, 2, 6], F32, tag="st", name="st")
            for sg in range(2):
                nc.vector.bn_stats(out=st[:, sg, :],
                                   in_=src_ap[:, sg * 512:(sg + 1) * 512])
            return st

        def ln_finish(sts, src_aps):
            n = len(src_aps)
            assert n <= 8
            mvs = statp.tile([128, 8, 2], F32, tag="mvs", name="mvs")
            for g, st in enumerate(sts):
                nc.vector.bn_aggr(out=mvs[:, g, :], in_=st)
            ve = statp.tile([128, 8], F32, tag="ve", name="ve")
            nc.vector.tensor_scalar_add(ve[:, :n], mvs[:, :n, 1], eps_b)
            iv = statp.tile([128, 8], mybir.dt.int32, tag="iv", name="iv")
            nc.vector.tensor_scalar(iv[:, :n], ve[:, :n].bitcast(mybir.dt.int32), 1,
                                    None, ALU.arith_shift_right)
            nc.vector.tensor_scalar(iv[:, :n], iv[:, :n], -1, 0x5F3759DF,
                                    ALU.mult, ALU.add)
            y = iv.bitcast(F32)
            u = statp.tile([128, 8], F32, tag="u", name="u")
            for _ in range(2):
                nc.vector.tensor_tensor(u[:, :n], y[:, :n], y[:, :n], ALU.mult)
                nc.vector.tensor_tensor(u[:, :n], u[:, :n], ve[:, :n], ALU.mult)
                nc.vector.tensor_scalar(u[:, :n], u[:, :n], -0.5, 1.5, ALU.mult, ALU.add)
                nc.vector.tensor_tensor(y[:, :n], y[:, :n], u[:, :n], ALU.mult)
            outs = []
            for g, src_ap in enumerate(src_aps):
                t = lnrmp.tile([128, C], F32R, tag="lnrm", name="lnt")
                nc.vector.tensor_scalar(t, src_ap, mvs[:, g, 0:1], y[:, g:g + 1],
                                        ALU.subtract, ALU.mult)
                outs.append(t)
            return outs

        def ln_group(src_aps):
            return ln_finish([ln_stats(a) for a in src_aps], src_aps)

        # transpose k row-tiles into feature-major fp8: dst [128, 8, W]
        def transposeN(srcs, dst_ap, W, evac=None):
            # srcs: row-tile APs [128, C] f32r; W = 128*len(srcs)
            nk = len(srcs)
            for ct in range(8):
                full = nk // 4
                for blk in range(full):
                    tp = psmm.tile([128, 512], F32R, tag="mm", name="trp")
                    for k in range(4):
                        nc.tensor.transpose(
                            tp[:, k * 128:(k + 1) * 128],
                            srcs[blk * 4 + k][:, ct * 128:(ct + 1) * 128], ident)
                    dst = dst_ap[:, ct, blk * 512:(blk + 1) * 512]
                    (evac or nc.vector.tensor_copy)(dst, tp)
                rem = nk - full * 4
                if rem:
                    tp = psmm.tile([128, 512], F32R, tag="mm", name="trp")
                    for k in range(rem):
                        nc.tensor.transpose(
                            tp[:, k * 128:(k + 1) * 128],
                            srcs[full * 4 + k][:, ct * 128:(ct + 1) * 128], ident)
                    dst = dst_ap[:, ct, full * 512:full * 512 + rem * 128]
                    (evac or nc.vector.tensor_copy)(dst, tp[:, 0:rem * 128])

        def col_block_dma(w_handle, o0, width=256):
            wc = wcolp.tile([128, KT, width], FP8, tag="wcol", name="wc")
            src = w_handle[:, o0:o0 + width].rearrange("(kt p) o -> p kt o", p=128)
            nc.sync.dma_start(out=wc, in_=src)
            return wc

        # DoubleRow GEMM: out^T tiles from weight col-blocks x rhs (fp8)
        def linearT(w_handle, o0_base, n_ot, rhs_fn, rhs_width, out_fn):
            for og in range(n_ot // 2):
                wc = col_block_dma(w_handle, o0_base + og * 256)
                for oi in range(2):
                    ot = og * 2 + oi
                    o_done = 0
                    kc = 0
                    while o_done < rhs_width:
                        cw = min(512, rhs_width - o_done)
                        pq = psmm.tile([128, 512], F32, tag="mm", name="pq")
                        for kp in range(KT // 2):
                            nc.tensor.matmul(
                                pq[:, 0:cw],
                                wc[:, 2 * kp:2 * kp + 2, oi * 128:(oi + 1) * 128],
                                rhs_fn(kp, kc, cw),
                                start=(kp == 0), stop=(kp == KT // 2 - 1),
                                perf_mode=DR)
                        out_fn(ot, kc, cw, pq)
                        o_done += cw
                        kc += 1

        # ---------------- attention (fp8 scores + DR attn@V + divide) -------
        def attention(qTt, kTt, Vt, msk, attnTt, expp, psS, psat):
            # head h = 4g + b4 lives in qT/kT tile t = h//2, half h%2.
            # attn@V for group g-1 is emitted interleaved into group g's
            # scores loop so PE has work while Act drains the exp chain.
            def mk_av(g, b4, exs):
                def f():
                    h = g * 4 + b4
                    pav = psat.tile([64, 512], F32, tag="pav", name="pav")
                    rpv = psat.tile([64, 512], F32, tag="rpv", name="rpv")
                    for kyp in range(3):
                        if kyp < 2:
                            nc.tensor.matmul(
                                pav, Vt[:, 2 * kyp:2 * kyp + 2, h, :],
                                exs[kyp][:, :, b4, :],
                                start=(kyp == 0), stop=False, perf_mode=DR)
                            nc.tensor.matmul(
                                rpv, onesV,
                                exs[kyp][:, :, b4, :],
                                start=(kyp == 0), stop=False, perf_mode=DR)
                        else:
                            nc.tensor.matmul(
                                pav, Vt[:, 4, h, :], exs[2][:, 0, b4, :],
                                start=False, stop=True)
                            nc.tensor.matmul(
                                rpv, onesV[:, 0, :], exs[2][:, 0, b4, :],
                                start=False, stop=True)
                    # no TT divide in the ISA: reciprocal to SBUF, then mult
                    rsb = finp.tile([64, 512], F32, tag="rsb", name="rsb")
                    nc.vector.reciprocal(out=rsb, in_=rpv)
                    dst = attnTt[(b4 % 2) * 64:(b4 % 2) * 64 + 64,
                                 2 * g + b4 // 2, :]
                    nc.vector.tensor_tensor(dst, pav, rsb, ALU.mult)
                return f

            pending = []
            for g in range(4):
                exs = []
                for ky in range(KYV):
                    kyp, par = divmod(ky, 2)
                    if par == 0:
                        ex = expp.tile([128, 2, 4, 512], FP8, tag="expS", name="ex")
                        exs.append(ex)
                    for hq in range(2):
                        ps = psS.tile([128, 1024], F32, tag="S", name="sps")
                        t = 2 * g + hq
                        for bi in range(2):
                            p0 = bi * 64
                            nc.tensor.matmul(
                                ps[:, bi * 512:(bi + 1) * 512],
                                kTt[p0:p0 + 64, t, ky * 128:(ky + 1) * 128],
                                qTt[p0:p0 + 64, t, :],
                                start=True, stop=True)
                        nc.scalar.activation(
                            exs[kyp][:, par, 2 * hq:2 * hq + 2, :], ps, AF.Exp,
                            bias=msk[:, ky:ky + 1], scale=SCL)
                        if pending:
                            pending.pop(0)()
                while pending:
                    pending.pop(0)()
                pending = [mk_av(g, b4, exs) for b4 in range(4)]
            while pending:
                pending.pop(0)()

        # V build: V = lnT^T W_v + 1 (x) b_v (rank-1 ones matmul carries the
        # bias into psum); evacuation on Act so DVE stays free for LN.
        def build_v(lhsT_tiles, bvrow, Vt, n_ky, w_handle=None, wv=None):
            if wv is None:
                wv = wcolp.tile([128, KT, C], FP8, tag="wrow", name="wv", bufs=1)
                nc.sync.dma_start(
                    out=wv, in_=w_handle.rearrange("(kt p) o -> p kt o", p=128))
            for ky in range(n_ky):
                for vc in range(2):
                    pv = psmm.tile([128, 512], F32, tag="mm", name="pv")
                    for kp in range(KT // 2):
                        nc.tensor.matmul(
                            pv, lhsT_tiles[:, 2 * kp:2 * kp + 2,
                                           ky * 128:(ky + 1) * 128],
                            wv[:, 2 * kp:2 * kp + 2, vc * 512:(vc + 1) * 512],
                            start=(kp == 0), stop=False,
                            perf_mode=DR)
                    nc.tensor.matmul(
                        pv, ones_r[0:1, 0:128],
                        bvrow[0:1, vc * 512:(vc + 1) * 512],
                        start=False, stop=True)
                    nc.scalar.activation(
                        Vt[:, ky, vc * 8:(vc + 1) * 8, :],
                        pv.rearrange("p (h d) -> p h d", h=8), AF.Identity)

        # helper: AP pair view for V attn lhsT [128, 2, 64] over ky dim
        # (implemented inline below; .pair2 used above is replaced)

        with tc.tile_pool(name="qTp", bufs=2) as qTp, \
             tc.tile_pool(name="kTp", bufs=2) as kTp, \
             tc.tile_pool(name="Vp", bufs=2) as Vp, \
             tc.tile_pool(name="attnTp", bufs=2) as attnTp, \
             tc.tile_pool(name="expp", bufs=5) as expp, \
             tc.tile_pool(name="psS", bufs=2, space="PSUM") as psS, \
             tc.tile_pool(name="psat", bufs=1, space="PSUM") as psat, \
             tc.tile_pool(name="cTp", bufs=1) as cTp, \
             tc.tile_pool(name="wf2p", bufs=1) as wf2p:

            # -------- Phase D first: ca kT, ca V depend only on cT -> fills
            # the PE-idle window while x is DMA'd and LN1 runs.
            cT_sb = cTp.tile([128, 8, NKV], FP8, tag="cT")
            nc.sync.dma_start(
                out=cT_sb, in_=cT_d.rearrange("(kt p) o -> p kt o", p=128))
            cakT = kTp.tile([128, 8, NKV], FP8, tag="kT", name="cakT")
            linearT(w_cak, 0, 8,
                    lambda kp, kc, cw: cT_sb[:, 2 * kp:2 * kp + 2,
                                             kc * 512:kc * 512 + cw], NKV,
                    lambda ot, kc, cw, pq: nc.scalar.activation(
                        cakT[:, ot, kc * 512:kc * 512 + cw], pq[:, 0:cw],
                        AF.Identity, bias=bcakT[:, ot:ot + 1]))
            caV = Vp.tile([128, KYV, 16, 64], FP8, tag="V", name="caV")
            build_v(cT_sb, bcavrow, caV, KYV, w_handle=w_cav)

            # -------- Phase A: LN1 for queries (own rows) and packed kv rows
            with tc.tile_pool(name="lnT1p", bufs=1) as lnT1p:
                ln1qT = lnT1p.tile([128, 8, 512], FP8, tag="lnTq")
                ln1kvT = lnT1p.tile([128, 8, NKV], FP8, tag="lnTkv")
                srcs_q = ln_group([x_sb[:, rt, :] for rt in range(4)])
                transposeN(srcs_q, ln1qT, 512)
                kv_srcs = []
                for rt in range(KYV):
                    xo = lnrmp.tile([128, C], BF16, tag="xoth", bufs=5, name="xo")
                    nc.gpsimd.dma_start(
                        out=xo, in_=x_kv[rt * 128:(rt + 1) * 128, :])
                    kv_srcs.append(xo)
                srcs_kv = ln_group(kv_srcs)
                transposeN(srcs_kv, ln1kvT, NKV)

                # -------- Phase B: sa qT, kT, V (DoubleRow fp8)
                qT = qTp.tile([128, 8, 512], FP8, tag="qT")
                linearT(w_qkv, 0, 8,
                        lambda kp, kc, cw: ln1qT[:, 2 * kp:2 * kp + 2, :], 512,
                        lambda ot, kc, cw, pq: nc.scalar.activation(
                            qT[:, ot, :], pq, AF.Identity,
                            bias=bqT[:, ot:ot + 1]))
                kT = kTp.tile([128, 8, NKV], FP8, tag="kT")
                linearT(w_qkv, C, 8,
                        lambda kp, kc, cw: ln1kvT[:, 2 * kp:2 * kp + 2,
                                                  kc * 512:kc * 512 + cw], NKV,
                        lambda ot, kc, cw, pq: nc.scalar.activation(
                            kT[:, ot, kc * 512:kc * 512 + cw], pq[:, 0:cw],
                            AF.Identity, bias=bkT[:, ot:ot + 1]))
                V = Vp.tile([128, KYV, 16, 64], FP8, tag="V")
                build_v(ln1kvT, bvrow, V, KYV, w_handle=w_qkv[:, 2048:3072])

            if True:
                # -------- Phase C: sa attention
                attnT = attnTp.tile([128, 8, 512], FP8, tag="attnT")
                attention(qT, kT, V, msk_sa, attnT, expp, psS, psat)

                # proj bias pre-add into x_sb (zeros in practice; overlaps attn)
                for rt in range(4):
                    nc.gpsimd.tensor_tensor(x_sb[:, rt, :], x_sb[:, rt, :],
                                            brep_proj, ALU.add)

                # -------- Phase E: sa proj + residual + LN2 (interleaved)
                def proj_residual(attnTt, w_handle, ln_after):
                    wp = wcolp.tile([128, KT, C], FP8, tag="wrow", name="wp",
                                    bufs=1)
                    nc.sync.dma_start(
                        out=wp,
                        in_=w_handle.rearrange("(kt p) o -> p kt o", p=128))
                    sts = []
                    for rt in range(4):
                        for oc in range(2):
                            pp = psmm.tile([128, 512], F32, tag="mm", name="pp")
                            for kp in range(KT // 2):
                                nc.tensor.matmul(
                                    pp,
                                    attnTt[:, 2 * kp:2 * kp + 2,
                                           rt * 128:(rt + 1) * 128],
                                    wp[:, 2 * kp:2 * kp + 2,
                                       oc * 512:(oc + 1) * 512],
                                    start=(kp == 0), stop=(kp == KT // 2 - 1),
                                    perf_mode=DR)
                            xsl = x_sb[:, rt, oc * 512:(oc + 1) * 512]
                            nc.vector.tensor_tensor(xsl, xsl, pp, ALU.add)
                        if ln_after:
                            sts.append(ln_stats(x_sb[:, rt, :]))
                    if ln_after:
                        return ln_finish(sts, [x_sb[:, r, :] for r in range(4)])
                    return None

                with tc.tile_pool(name="lnT2p", bufs=1) as lnT2p:
                    ln2T = lnT2p.tile([128, 8, 512], FP8, tag="lnT2")
                    lns2 = proj_residual(attnT, w_proj, True)
                    transposeN(lns2, ln2T, 512)

                    # -------- Phase F: ca qT
                    caqT = qTp.tile([128, 8, 512], FP8, tag="qT", name="caqT")
                    linearT(w_caq, 0, 8,
                            lambda kp, kc, cw: ln2T[:, 2 * kp:2 * kp + 2, :], 512,
                            lambda ot, kc, cw, pq: nc.scalar.activation(
                                caqT[:, ot, :], pq, AF.Identity,
                                bias=bcaqT[:, ot:ot + 1]))

                # prefetch fc2 hi weights now (Pool DMA queue; used ~60us later)
                wf2h = wf2p.tile([128, 32, C], FP8, tag="wf2h")
                nc.gpsimd.dma_start(
                    out=wf2h, in_=w_fc2h.rearrange("(kt p) o -> p kt o", p=128))

                # -------- Phase G: ca attention
                caattnT = attnTp.tile([128, 8, 512], FP8, tag="attnT",
                                      name="caattnT")
                attention(caqT, cakT, caV, msk_ca, caattnT, expp, psS, psat)

                for rt in range(4):
                    nc.gpsimd.tensor_tensor(x_sb[:, rt, :], x_sb[:, rt, :],
                                            brep_caproj, ALU.add)

                # -------- Phase H: ca proj + residual + LN3 stats
                lns3 = proj_residual(caattnT, w_caproj, True)

        # ---------------- MLP scope ----------------
        with tc.tile_pool(name="mlpp", bufs=1) as mlpp, \
             tc.tile_pool(name="wfc2p", bufs=3) as wfc2p, \
             tc.tile_pool(name="pfp", bufs=2, space="PSUM") as pfp:
            # ln3 in hi+lo fp8 (residual quantization) to kill x-hat quant err
            ln3T = mlpp.tile([128, 8, 512], FP8, tag="lnT3")
            ln3lT = mlpp.tile([128, 8, 512], FP8, tag="lnT3l")
            for ct in range(8):
                tp = psmm.tile([128, 512], F32R, tag="mm", name="trp")
                for k in range(4):
                    nc.tensor.transpose(tp[:, k * 128:(k + 1) * 128],
                                        lns3[k][:, ct * 128:(ct + 1) * 128],
                                        ident)
                nc.scalar.activation(ln3T[:, ct, :], tp.bitcast(F32),
                                      AF.Identity)
                nc.vector.tensor_tensor(ln3lT[:, ct, :], tp.bitcast(F32),
                                        ln3T[:, ct, :], ALU.subtract)
            x3T = mlpp.tile([128, 8, 512], F32, tag="x3T")
            for ct in range(8):
                tp = psmm.tile([128, 512], F32, tag="mm", name="trp3")
                for k in range(4):
                    nc.tensor.transpose(tp[:, k * 128:(k + 1) * 128],
                                        x_sb[:, k, ct * 128:(ct + 1) * 128],
                                        ident32)
                # fold fc2 bias into x3T while evacuating
                nc.scalar.activation(x3T[:, ct, :], tp, AF.Identity,
                                     bias=bfc2T[:, ct:ct + 1])

            h1gT = mlpp.tile([128, 32, 512], FP8, tag="h1gT")
            for og in range(16):
                wch = col_block_dma(w_fc1h, og * 256)
                wcl = col_block_dma(w_fc1l, og * 256)
                for oi in range(2):
                    ot = og * 2 + oi
                    pf = pfp.tile([128, 512], F32, tag="pf", name="pf")
                    passes = [(wch, ln3T), (wcl, ln3T), (wch, ln3lT)]
                    for pi, (wc_, rhs) in enumerate(passes):
                        for kp in range(KT // 2):
                            nc.tensor.matmul(
                                pf,
                                wc_[:, 2 * kp:2 * kp + 2, oi * 128:(oi + 1) * 128],
                                rhs[:, 2 * kp:2 * kp + 2, :],
                                start=(pi == 0 and kp == 0),
                                stop=(pi == 2 and kp == KT // 2 - 1),
                                perf_mode=DR)
                    nc.scalar.activation(h1gT[:, ot, :], pf, AF.Gelu,
                                         bias=bfc1T[:, ot:ot + 1], scale=1.0 / WS)

            for ot in range(8):
                wfl = wfc2p.tile([128, 32, 128], FP8, tag="wfc2", name="wfl")
                nc.gpsimd.dma_start(
                    out=wfl, in_=w_fc2l[:, ot * 128:(ot + 1) * 128].rearrange(
                        "(kt p) o -> p kt o", p=128))
                pm = psmm.tile([128, 512], F32, tag="mm", name="pm")
                for kp in range(16):
                    nc.tensor.matmul(
                        pm, wf2h[:, 2 * kp:2 * kp + 2, ot * 128:(ot + 1) * 128],
                        h1gT[:, 2 * kp:2 * kp + 2, :],
                        start=(kp == 0), stop=False, perf_mode=DR)
                for kp in range(16):
                    nc.tensor.matmul(pm, wfl[:, 2 * kp:2 * kp + 2, :],
                                     h1gT[:, 2 * kp:2 * kp + 2, :],
                                     start=False, stop=(kp == 15),
                                     perf_mode=DR)
                fin = finp.tile([128, 512], F32, tag="fin", name="fin")
                nc.scalar.activation(fin, pm, AF.Identity, scale=1.0 / WS)
                nc.vector.tensor_tensor(fin, fin, x3T[:, ot, :], ALU.add)
                nc.sync.dma_start(out=yT[ot * 128:(ot + 1) * 128, :], in_=fin)

    nc.finalize()
    return nc, dbg


# ---------------- host-side prep ----------------

def _prep_inputs(i, x, c, mask, sa_qkv_w, sa_qkv_b, sa_proj_w, sa_proj_b,
                 ca_q_w, ca_q_b, ca_k_w, ca_k_b, ca_v_w, ca_v_b,
                 ca_proj_w, ca_proj_b, fc1_w, fc1_b, fc2_w, fc2_b):
    import ml_dtypes
    f8 = ml_dtypes.float8_e4m3
    f = np.float32
    b_i, rh = i // 2, i % 2
    r0 = rh * R

    def colT(vec, n):  # bias vector -> [128, n] transposed col layout
        return np.ascontiguousarray(np.asarray(vec, f).reshape(n, 128).T)

    # pack keys: indices of unmasked rows of this batch
    mb = np.asarray(mask[b_i])
    kv_idx = np.nonzero(mb == 1)[0]
    assert len(kv_idx) <= NKV, f"unmasked count {len(kv_idx)} > NKV={NKV}"
    nkv = len(kv_idx)
    x_kv = np.zeros((NKV, C), ml_dtypes.bfloat16)
    x_kv[:nkv] = np.asarray(x[b_i], f)[kv_idx].astype(ml_dtypes.bfloat16)
    c_kv = np.zeros((NKV, C), f)
    c_kv[:nkv] = np.asarray(c[b_i], f)[kv_idx]
    mrow = np.full(NKV, -10000.0, f)
    mrow[:nkv] = 0.0
    mcol = np.ascontiguousarray(mrow.reshape(KYV, 128).T)

    w1s = np.asarray(fc1_w, f) * WS
    w1h = w1s.astype(f8)
    w2s = np.asarray(fc2_w, f) * WS
    w2h = w2s.astype(f8)
    wq = np.asarray(sa_qkv_w[:, 0:C], f) * WS
    wk = np.asarray(sa_qkv_w[:, C:2 * C], f) * WS
    wv = np.asarray(sa_qkv_w[:, 2 * C:3 * C], f)
    w_qkv = np.concatenate([wq, wk, wv], axis=1).astype(f8)
    bq = np.asarray(sa_qkv_b[0:C], f) * WS
    bk = np.asarray(sa_qkv_b[C:2 * C], f) * WS

    return {
        "x_own": np.ascontiguousarray(np.asarray(x[b_i], f)[r0:r0 + R]),
        "x_kv": x_kv,
        "cT": np.ascontiguousarray(c_kv.T).astype(f8),
        "mask_sa": mcol,
        "mask_ca": mcol,
        "w_qkv": w_qkv,
        "w_proj": np.asarray(sa_proj_w, f).astype(f8),
        "w_caq": (np.asarray(ca_q_w, f) * WS).astype(f8),
        "w_cak": (np.asarray(ca_k_w, f) * WS).astype(f8),
        "w_cav": np.asarray(ca_v_w, f).astype(f8),
        "w_caproj": np.asarray(ca_proj_w, f).astype(f8),
        "w_fc1h": w1h,
        "w_fc1l": (w1s - w1h.astype(f)).astype(f8),
        "w_fc2h": w2h,
        "w_fc2l": (w2s - w2h.astype(f)).astype(f8),
        "b_qT": colT(bq, 8),
        "b_kT": colT(bk, 8),
        "b_caqT": colT(np.asarray(ca_q_b, f) * WS, 8),
        "b_cakT": colT(np.asarray(ca_k_b, f) * WS, 8),
        "b_fc1T": colT(np.asarray(fc1_b, f) * WS, 32),
        "b_fc2T": colT(fc2_b, 8),
        "b_vr": np.asarray(sa_qkv_b[2 * C:3 * C], f).reshape(1, C),
        "b_cavr": np.asarray(ca_v_b, f).reshape(1, C),
        "b_projr": np.asarray(sa_proj_b, f).reshape(1, C),
        "b_caprojr": np.asarray(ca_proj_b, f).reshape(1, C),
    }


def kernel(**inputs):
    inputs = {k: np.asarray(v) for k, v in inputs.items()}
    if "prog" not in _cache:
        _cache["prog"] = build_program()[0]
    nc = _cache["prog"]
    in_maps = [_prep_inputs(i, **inputs) for i in range(8)]
    res = run_bass_kernel_spmd(nc, in_maps, core_ids=list(range(8)))
    out = np.empty((B, N, C), np.float32)
    for i in range(8):
        b_i, rh = i // 2, i % 2
        out[b_i, rh * R:(rh + 1) * R, :] = res.results[i]["yT"].T
    return out


# revision 38
# speedup vs baseline: 1.0038x; 1.0038x over previous
"""Trainium2 Bass kernel for nn_DiTBlock (B=4,N=1024,C=1024,H=16).

8-way SPMD: core i handles batch i//2, row-half i%2 (512 query rows).

- All big GEMMs run as fp8e4 DoubleRow matmuls (2 K-tiles/instruction,
  0.5 cyc/row = 4x bf16); scores and attn@V stay fp8 (attn@V DoubleRow
  over key-tile pairs). MLP accuracy is recovered with hi+lo fp8
  residual splits (fc1: 3 passes incl. an x-hat lo pass; fc2: 2 passes),
  keeping rel err ~1.2e-2 vs the 2e-2 gate.
- Keys are host-packed to the unmasked subset padded to NKV=640 (the
  mask is a kernel input, so packing is host-side work), shrinking
  exp/scores/attn@V/kT/V by ~40%.
- Softmax in S^T layout: mask rides the exp bias per partition, the
  denominator comes from an all-ones fp8 DoubleRow matmul, normalize =
  DVE reciprocal + multiply (no ISA divide).
- q/k/fc1 weights are stored x32 in fp8 to avoid subnormals; the exp /
  gelu activation scale compensates.
- Scheduling: ca k/v GEMMs run first (fill LN1 window), attn@V of group
  g-1 interleaves into group g's scores so PE works during the exp
  chain, per-engine queues balanced (exp/gelu/evac on Act, LN + divides
  on DVE, SBUF-only work on Pool), fc2 hi-weights prefetched during ca
  attention.
"""
import numpy as np
from contextlib import ExitStack

import concourse.bass as bass
import concourse.bacc as bacc
import concourse.mybir as mybir
import concourse.tile as tile
from concourse.bass_utils import run_bass_kernel_spmd
from concourse.masks import make_identity

F32 = mybir.dt.float32
F32R = mybir.dt.float32r
BF16 = mybir.dt.bfloat16
FP8 = mybir.dt.float8e4
AF = mybir.ActivationFunctionType
ALU = mybir.AluOpType
DR = mybir.MatmulPerfMode.DoubleRow

B, N, C, H, D = 4, 1024, 1024, 16, 64
HID = 4 * C
R = 512            # own query rows per core
NKV = 640          # packed+padded key count (binomial(1024,.5) <= 640 at 8 sigma)
KYV = NKV // 128   # 5 key tiles
KT = C // 128      # 8
EPS = 1e-6
WS = 32.0          # fp8 weight prescale for q/k/fc1
SCL = 0.125 / (WS * WS)   # exp scale compensating q~ = 32q, k~ = 32k

_cache = {}


def build_program(debug=False):
    nc = bacc.Bacc(None, target_bir_lowering=False)
    dbg = {}

    # ---------------- DRAM handles ----------------
    x_own = nc.dram_tensor("x_own", [R, C], F32, kind="ExternalInput")
    x_kv = nc.dram_tensor("x_kv", [NKV, C], BF16, kind="ExternalInput")
    cT_d = nc.dram_tensor("cT", [C, NKV], FP8, kind="ExternalInput")
    mask_sa = nc.dram_tensor("mask_sa", [128, KYV], F32, kind="ExternalInput")
    mask_ca = nc.dram_tensor("mask_ca", [128, KYV], F32, kind="ExternalInput")
    # weights (fp8; q/k cols head-permuted and x32, v cols plain)
    w_qkv = nc.dram_tensor("w_qkv", [C, 3 * C], FP8, kind="ExternalInput")
    w_proj = nc.dram_tensor("w_proj", [C, C], FP8, kind="ExternalInput")
    w_caq = nc.dram_tensor("w_caq", [C, C], FP8, kind="ExternalInput")
    w_cak = nc.dram_tensor("w_cak", [C, C], FP8, kind="ExternalInput")
    w_cav = nc.dram_tensor("w_cav", [C, C], FP8, kind="ExternalInput")
    w_caproj = nc.dram_tensor("w_caproj", [C, C], FP8, kind="ExternalInput")
    w_fc1h = nc.dram_tensor("w_fc1h", [C, HID], FP8, kind="ExternalInput")
    w_fc1l = nc.dram_tensor("w_fc1l", [C, HID], FP8, kind="ExternalInput")
    w_fc2h = nc.dram_tensor("w_fc2h", [HID, C], FP8, kind="ExternalInput")
    w_fc2l = nc.dram_tensor("w_fc2l", [HID, C], FP8, kind="ExternalInput")
    # biases: transposed column layouts [128, n_tiles] f32 (q/k x32+perm)
    b_qT = nc.dram_tensor("b_qT", [128, 8], F32, kind="ExternalInput")
    b_kT = nc.dram_tensor("b_kT", [128, 8], F32, kind="ExternalInput")
    b_caqT = nc.dram_tensor("b_caqT", [128, 8], F32, kind="ExternalInput")
    b_cakT = nc.dram_tensor("b_cakT", [128, 8], F32, kind="ExternalInput")
    b_fc1T = nc.dram_tensor("b_fc1T", [128, 32], F32, kind="ExternalInput")
    b_fc2T = nc.dram_tensor("b_fc2T", [128, 8], F32, kind="ExternalInput")
    # replicated-row biases [1, C] f32 (broadcast-DMA'd)
    b_vr = nc.dram_tensor("b_vr", [1, C], F32, kind="ExternalInput")
    b_cavr = nc.dram_tensor("b_cavr", [1, C], F32, kind="ExternalInput")
    b_projr = nc.dram_tensor("b_projr", [1, C], F32, kind="ExternalInput")
    b_caprojr = nc.dram_tensor("b_caprojr", [1, C], F32, kind="ExternalInput")
    yT = nc.dram_tensor("yT", [C, R], F32, kind="ExternalOutput")

    with tile.TileContext(nc) as tc, ExitStack() as ctx:
        misc = ctx.enter_context(tc.tile_pool(name="misc", bufs=1))
        pxp = ctx.enter_context(tc.tile_pool(name="pxp", bufs=1))
        lnrmp = ctx.enter_context(tc.tile_pool(name="lnrmp", bufs=5))
        statp = ctx.enter_context(tc.tile_pool(name="statp", bufs=6))
        finp = ctx.enter_context(tc.tile_pool(name="finp", bufs=2))
        wcolp = ctx.enter_context(tc.tile_pool(name="wcolp", bufs=6))
        psmm = ctx.enter_context(tc.tile_pool(name="psmm", bufs=2, space="PSUM"))

        x_sb = pxp.tile([128, 4, C], F32, tag="x")
        for rt in range(4):
            nc.gpsimd.dma_start(out=x_sb[:, rt, :],
                                in_=x_own[rt * 128:(rt + 1) * 128, :])

        ident32 = misc.tile([128, 128], F32)
        make_identity(nc, ident32)
        ident = misc.tile([128, 128], F32R)
        nc.vector.tensor_copy(ident, ident32)
        onesV = misc.tile([128, 2, 64], FP8)
        nc.vector.memset(onesV, 1.0)
        ones_r = misc.tile([1, 128], BF16)
        nc.gpsimd.memset(ones_r, 1.0)
        eps_b = misc.tile([128, 1], F32)
        nc.gpsimd.memset(eps_b, EPS)
        msk_sa = misc.tile([128, KYV], F32)
        nc.gpsimd.dma_start(out=msk_sa, in_=mask_sa[:, :])
        msk_ca = misc.tile([128, KYV], F32)
        nc.gpsimd.dma_start(out=msk_ca, in_=mask_ca[:, :])
        bqT = misc.tile([128, 8], F32)
        nc.gpsimd.dma_start(out=bqT, in_=b_qT[:, :])
        bkT = misc.tile([128, 8], F32)
        nc.gpsimd.dma_start(out=bkT, in_=b_kT[:, :])
        bcaqT = misc.tile([128, 8], F32)
        nc.gpsimd.dma_start(out=bcaqT, in_=b_caqT[:, :])
        bcakT = misc.tile([128, 8], F32)
        nc.gpsimd.dma_start(out=bcakT, in_=b_cakT[:, :])
        bfc1T = misc.tile([128, 32], F32)
        nc.gpsimd.dma_start(out=bfc1T, in_=b_fc1T[:, :])
        bfc2T = misc.tile([128, 8], F32)
        nc.gpsimd.dma_start(out=bfc2T, in_=b_fc2T[:, :])

        def bcast_load(dst, src_handle):
            s = src_handle[0:1, :]
            ap = bass.AP(tensor=s.tensor, offset=s.offset, ap=[[0, 128], [1, C]])
            nc.gpsimd.dma_start(out=dst, in_=ap)

        bvrow = misc.tile([1, C], BF16)
        nc.gpsimd.dma_start(out=bvrow, in_=b_vr[0:1, :])
        bcavrow = misc.tile([1, C], BF16)
        nc.gpsimd.dma_start(out=bcavrow, in_=b_cavr[0:1, :])
        brep_proj = misc.tile([128, C], F32)
        bcast_load(brep_proj, b_projr)
        brep_caproj = misc.tile([128, C], F32)
        bcast_load(brep_caproj, b_caprojr)

        # ---------------- LN helpers (baseline bn_stats + rsqrt bit-trick) --
        def ln_stats(src_ap):
            st = statp.tile([128, 2, 6], F32, tag="st", name="st")
            for sg in range(2):
                nc.vector.bn_stats(out=st[:, sg, :],
                                   in_=src_ap[:, sg * 512:(sg + 1) * 512])
            return st

        def ln_finish(sts, src_aps):
            n = len(src_aps)
            assert n <= 8
            mvs = statp.tile([128, 8, 2], F32, tag="mvs", name="mvs")
            for g, st in enumerate(sts):
                nc.vector.bn_aggr(out=mvs[:, g, :], in_=st)
            ve = statp.tile([128, 8], F32, tag="ve", name="ve")
            nc.vector.tensor_scalar_add(ve[:, :n], mvs[:, :n, 1], eps_b)
            iv = statp.tile([128, 8], mybir.dt.int32, tag="iv", name="iv")
            nc.vector.tensor_scalar(iv[:, :n], ve[:, :n].bitcast(mybir.dt.int32), 1,
                                    None, ALU.arith_shift_right)
            nc.vector.tensor_scalar(iv[:, :n], iv[:, :n], -1, 0x5F3759DF,
                                    ALU.mult, ALU.add)
            y = iv.bitcast(F32)
            u = statp.tile([128, 8], F32, tag="u", name="u")
            for _ in range(2):
                nc.vector.tensor_tensor(u[:, :n], y[:, :n], y[:, :n], ALU.mult)
                nc.vector.tensor_tensor(u[:, :n], u[:, :n], ve[:, :n], ALU.mult)
                nc.vector.tensor_scalar(u[:, :n], u[:, :n], -0.5, 1.5, ALU.mult, ALU.add)
                nc.vector.tensor_tensor(y[:, :n], y[:, :n], u[:, :n], ALU.mult)
            outs = []
            for g, src_ap in enumerate(src_aps):
                t = lnrmp.tile([128, C], F32R, tag="lnrm", name="lnt")
                nc.vector.tensor_scalar(t, src_ap, mvs[:, g, 0:1], y[:, g:g + 1],
                                        ALU.subtract, ALU.mult)
                outs.append(t)
            return outs

        def ln_group(src_aps):
            return ln_finish([ln_stats(a) for a in src_aps], src_aps)

        # transpose k row-tiles into feature-major fp8: dst [128, 8, W]
        def transposeN(srcs, dst_ap, W, evac=None):
            # srcs: row-tile APs [128, C] f32r; W = 128*len(srcs)
            nk = len(srcs)
            for ct in range(8):
                full = nk // 4
                for blk in range(full):
                    tp = psmm.tile([128, 512], F32R, tag="mm", name="trp")
                    for k in range(4):
                        nc.tensor.transpose(
                            tp[:, k * 128:(k + 1) * 128],
                            srcs[blk * 4 + k][:, ct * 128:(ct + 1) * 128], ident)
                    dst = dst_ap[:, ct, blk * 512:(blk + 1) * 512]
                    (evac or nc.vector.tensor_copy)(dst, tp)
                rem = nk - full * 4
                if rem:
                    tp = psmm.tile([128, 512], F32R, tag="mm", name="trp")
                    for k in range(rem):
                        nc.tensor.transpose(
                            tp[:, k * 128:(k + 1) * 128],
                            srcs[full * 4 + k][:, ct * 128:(ct + 1) * 128], ident)
                    dst = dst_ap[:, ct, full * 512:full * 512 + rem * 128]
                    (evac or nc.vector.tensor_copy)(dst, tp[:, 0:rem * 128])

        def col_block_dma(w_handle, o0, width=256):
            wc = wcolp.tile([128, KT, width], FP8, tag="wcol", name="wc")
            src = w_handle[:, o0:o0 + width].rearrange("(kt p) o -> p kt o", p=128)
            nc.sync.dma_start(out=wc, in_=src)
            return wc

        # DoubleRow GEMM: out^T tiles from weight col-blocks x rhs (fp8)
        def linearT(w_handle, o0_base, n_ot, rhs_fn, rhs_width, out_fn):
            for og in range(n_ot // 2):
                wc = col_block_dma(w_handle, o0_base + og * 256)
                for oi in range(2):
                    ot = og * 2 + oi
                    o_done = 0
                    kc = 0
                    while o_done < rhs_width:
                        cw = min(512, rhs_width - o_done)
                        pq = psmm.tile([128, 512], F32, tag="mm", name="pq")
                        for kp in range(KT // 2):
                            nc.tensor.matmul(
                                pq[:, 0:cw],
                                wc[:, 2 * kp:2 * kp + 2, oi * 128:(oi + 1) * 128],
                                rhs_fn(kp, kc, cw),
                                start=(kp == 0), stop=(kp == KT // 2 - 1),
                                perf_mode=DR)
                        out_fn(ot, kc, cw, pq)
                        o_done += cw
                        kc += 1

        # ---------------- attention (fp8 scores + DR attn@V + divide) -------
        def attention(qTt, kTt, Vt, msk, attnTt, expp, psS, psat):
            # head h = 4g + b4 lives in qT/kT tile t = h//2, half h%2.
            # attn@V for group g-1 is emitted interleaved into group g's
            # scores loop so PE has work while Act drains the exp chain.
            def mk_av(g, b4, exs):
                def f():
                    h = g * 4 + b4
                    pav = psat.tile([64, 512], F32, tag="pav", name="pav")
                    rpv = psat.tile([64, 512], F32, tag="rpv", name="rpv")
                    for kyp in range(3):
                        if kyp < 2:
                            nc.tensor.matmul(
                                pav, Vt[:, 2 * kyp:2 * kyp + 2, h, :],
                                exs[kyp][:, :, b4, :],
                                start=(kyp == 0), stop=False, perf_mode=DR)
                            nc.tensor.matmul(
                                rpv, onesV,
                                exs[kyp][:, :, b4, :],
                                start=(kyp == 0), stop=False, perf_mode=DR)
                        else:
                            nc.tensor.matmul(
                                pav, Vt[:, 4, h, :], exs[2][:, 0, b4, :],
                                start=False, stop=True)
                            nc.tensor.matmul(
                                rpv, onesV[:, 0, :], exs[2][:, 0, b4, :],
                                start=False, stop=True)
                    # no TT divide in the ISA: reciprocal to SBUF, then mult
                    rsb = finp.tile([64, 512], F32, tag="rsb", name="rsb")
                    nc.vector.reciprocal(out=rsb, in_=rpv)
                    dst = attnTt[(b4 % 2) * 64:(b4 % 2) * 64 + 64,
                                 2 * g + b4 // 2, :]
                    nc.vector.tensor_tensor(dst, pav, rsb, ALU.mult)
                return f

            pending = []
            for g in range(4):
                exs = []
                for ky in range(KYV):
                    kyp, par = divmod(ky, 2)
                    if par == 0:
                        ex = expp.tile([128, 2, 4, 512], FP8, tag="expS", name="ex")
                        exs.append(ex)
                    for hq in range(2):
                        ps = psS.tile([128, 1024], F32, tag="S", name="sps")
                        t = 2 * g + hq
                        for bi in range(2):
                            p0 = bi * 64
                            nc.tensor.matmul(
                                ps[:, bi * 512:(bi + 1) * 512],
                                kTt[p0:p0 + 64, t, ky * 128:(ky + 1) * 128],
                                qTt[p0:p0 + 64, t, :],
                                start=True, stop=True)
                        nc.scalar.activation(
                            exs[kyp][:, par, 2 * hq:2 * hq + 2, :], ps, AF.Exp,
                            bias=msk[:, ky:ky + 1], scale=SCL)
                        if pending:
                            pending.pop(0)()
                while pending:
                    pending.pop(0)()
                pending = [mk_av(g, b4, exs) for b4 in range(4)]
            while pending:
                pending.pop(0)()

        # V build: V = lnT^T W_v + 1 (x) b_v (rank-1 ones matmul carries the
        # bias into psum); evacuation on Act so DVE stays free for LN.
        def build_v(lhsT_tiles, bvrow, Vt, n_ky, w_handle=None, wv=None):
            if wv is None:
                wv = wcolp.tile([128, KT, C], FP8, tag="wrow", name="wv", bufs=1)
                nc.sync.dma_start(
                    out=wv, in_=w_handle.rearrange("(kt p) o -> p kt o", p=128))
            for ky in range(n_ky):
                for vc in range(2):
                    pv = psmm.tile([128, 512], F32, tag="mm", name="pv")
                    for kp in range(KT // 2):
                        nc.tensor.matmul(
                            pv, lhsT_tiles[:, 2 * kp:2 * kp + 2,
                                           ky * 128:(ky + 1) * 128],
                            wv[:, 2 * kp:2 * kp + 2, vc * 512:(vc + 1) * 512],
                            start=(kp == 0), stop=False,
                            perf_mode=DR)
                    nc.tensor.matmul(
                        pv, ones_r[0:1, 0:128],
                        bvrow[0:1, vc * 512:(vc + 1) * 512],
                        start=False, stop=True)
                    nc.scalar.activation(
                        Vt[:, ky, vc * 8:(vc + 1) * 8, :],
                        pv.rearrange("p (h d) -> p h d", h=8), AF.Identity)

        # helper: AP pair view for V attn lhsT [128, 2, 64] over ky dim
        # (implemented inline below; .pair2 used above is replaced)

        with tc.tile_pool(name="qTp", bufs=2) as qTp, \
             tc.tile_pool(name="kTp", bufs=2) as kTp, \
             tc.tile_pool(name="Vp", bufs=2) as Vp, \
             tc.tile_pool(name="attnTp", bufs=2) as attnTp, \
             tc.tile_pool(name="expp", bufs=6) as expp, \
             tc.tile_pool(name="psS", bufs=2, space="PSUM") as psS, \
             tc.tile_pool(name="psat", bufs=1, space="PSUM") as psat, \
             tc.tile_pool(name="cTp", bufs=1) as cTp, \
             tc.tile_pool(name="wf2p", bufs=1) as wf2p:

            # -------- Phase D first: ca kT, ca V depend only on cT -> fills
            # the PE-idle window while x is DMA'd and LN1 runs.
            cT_sb = cTp.tile([128, 8, NKV], FP8, tag="cT")
            nc.sync.dma_start(
                out=cT_sb, in_=cT_d.rearrange("(kt p) o -> p kt o", p=128))
            cakT = kTp.tile([128, 8, NKV], FP8, tag="kT", name="cakT")
            linearT(w_cak, 0, 8,
                    lambda kp, kc, cw: cT_sb[:, 2 * kp:2 * kp + 2,
                                             kc * 512:kc * 512 + cw], NKV,
                    lambda ot, kc, cw, pq: nc.scalar.activation(
                        cakT[:, ot, kc * 512:kc * 512 + cw], pq[:, 0:cw],
                        AF.Identity, bias=bcakT[:, ot:ot + 1]))
            caV = Vp.tile([128, KYV, 16, 64], FP8, tag="V", name="caV")
            build_v(cT_sb, bcavrow, caV, KYV, w_handle=w_cav)

            # -------- Phase A: LN1 for queries (own rows) and packed kv rows
            with tc.tile_pool(name="lnT1p", bufs=1) as lnT1p:
                ln1qT = lnT1p.tile([128, 8, 512], FP8, tag="lnTq")
                ln1kvT = lnT1p.tile([128, 8, NKV], FP8, tag="lnTkv")
                srcs_q = ln_group([x_sb[:, rt, :] for rt in range(4)])
                transposeN(srcs_q, ln1qT, 512)
                kv_srcs = []
                for rt in range(KYV):
                    xo = lnrmp.tile([128, C], BF16, tag="xoth", bufs=5, name="xo")
                    nc.gpsimd.dma_start(
                        out=xo, in_=x_kv[rt * 128:(rt + 1) * 128, :])
                    kv_srcs.append(xo)
                srcs_kv = ln_group(kv_srcs)
                transposeN(srcs_kv, ln1kvT, NKV)

                # -------- Phase B: sa qT, kT, V (DoubleRow fp8)
                qT = qTp.tile([128, 8, 512], FP8, tag="qT")
                linearT(w_qkv, 0, 8,
                        lambda kp, kc, cw: ln1qT[:, 2 * kp:2 * kp + 2, :], 512,
                        lambda ot, kc, cw, pq: nc.scalar.activation(
                            qT[:, ot, :], pq, AF.Identity,
                            bias=bqT[:, ot:ot + 1]))
                kT = kTp.tile([128, 8, NKV], FP8, tag="kT")
                linearT(w_qkv, C, 8,
                        lambda kp, kc, cw: ln1kvT[:, 2 * kp:2 * kp + 2,
                                                  kc * 512:kc * 512 + cw], NKV,
                        lambda ot, kc, cw, pq: nc.scalar.activation(
                            kT[:, ot, kc * 512:kc * 512 + cw], pq[:, 0:cw],
                            AF.Identity, bias=bkT[:, ot:ot + 1]))
                V = Vp.tile([128, KYV, 16, 64], FP8, tag="V")
                build_v(ln1kvT, bvrow, V, KYV, w_handle=w_qkv[:, 2048:3072])

            if True:
                # -------- Phase C: sa attention
                attnT = attnTp.tile([128, 8, 512], FP8, tag="attnT")
                attention(qT, kT, V, msk_sa, attnT, expp, psS, psat)

                # proj bias pre-add into x_sb (zeros in practice; overlaps attn)
                for rt in range(4):
                    nc.gpsimd.tensor_tensor(x_sb[:, rt, :], x_sb[:, rt, :],
                                            brep_proj, ALU.add)

                # -------- Phase E: sa proj + residual + LN2 (interleaved)
                def proj_residual(attnTt, w_handle, ln_after):
                    wp = wcolp.tile([128, KT, C], FP8, tag="wrow", name="wp",
                                    bufs=1)
                    nc.sync.dma_start(
                        out=wp,
                        in_=w_handle.rearrange("(kt p) o -> p kt o", p=128))
                    sts = []
                    for rt in range(4):
                        for oc in range(2):
                            pp = psmm.tile([128, 512], F32, tag="mm", name="pp")
                            for kp in range(KT // 2):
                                nc.tensor.matmul(
                                    pp,
                                    attnTt[:, 2 * kp:2 * kp + 2,
                                           rt * 128:(rt + 1) * 128],
                                    wp[:, 2 * kp:2 * kp + 2,
                                       oc * 512:(oc + 1) * 512],
                                    start=(kp == 0), stop=(kp == KT // 2 - 1),
                                    perf_mode=DR)
                            xsl = x_sb[:, rt, oc * 512:(oc + 1) * 512]
                            nc.vector.tensor_tensor(xsl, xsl, pp, ALU.add)
                        if ln_after:
                            sts.append(ln_stats(x_sb[:, rt, :]))
                    if ln_after:
                        return ln_finish(sts, [x_sb[:, r, :] for r in range(4)])
                    return None

                with tc.tile_pool(name="lnT2p", bufs=1) as lnT2p:
                    ln2T = lnT2p.tile([128, 8, 512], FP8, tag="lnT2")
                    lns2 = proj_residual(attnT, w_proj, True)
                    transposeN(lns2, ln2T, 512)

                    # -------- Phase F: ca qT
                    caqT = qTp.tile([128, 8, 512], FP8, tag="qT", name="caqT")
                    linearT(w_caq, 0, 8,
                            lambda kp, kc, cw: ln2T[:, 2 * kp:2 * kp + 2, :], 512,
                            lambda ot, kc, cw, pq: nc.scalar.activation(
                                caqT[:, ot, :], pq, AF.Identity,
                                bias=bcaqT[:, ot:ot + 1]))

                # prefetch fc2 hi weights now (Pool DMA queue; used ~60us later)
                wf2h = wf2p.tile([128, 32, C], FP8, tag="wf2h")
                nc.gpsimd.dma_start(
                    out=wf2h, in_=w_fc2h.rearrange("(kt p) o -> p kt o", p=128))

                # -------- Phase G: ca attention
                caattnT = attnTp.tile([128, 8, 512], FP8, tag="attnT",
                                      name="caattnT")
                attention(caqT, cakT, caV, msk_ca, caattnT, expp, psS, psat)

                for rt in range(4):
                    nc.gpsimd.tensor_tensor(x_sb[:, rt, :], x_sb[:, rt, :],
                                            brep_caproj, ALU.add)

                # -------- Phase H: ca proj + residual + LN3 stats
                lns3 = proj_residual(caattnT, w_caproj, True)

        # ---------------- MLP scope ----------------
        with tc.tile_pool(name="mlpp", bufs=1) as mlpp, \
             tc.tile_pool(name="wfc2p", bufs=3) as wfc2p, \
             tc.tile_pool(name="pfp", bufs=2, space="PSUM") as pfp:
            # ln3 in hi+lo fp8 (residual quantization) to kill x-hat quant err
            ln3T = mlpp.tile([128, 8, 512], FP8, tag="lnT3")
            ln3lT = mlpp.tile([128, 8, 512], FP8, tag="lnT3l")
            for ct in range(8):
                tp = psmm.tile([128, 512], F32R, tag="mm", name="trp")
                for k in range(4):
                    nc.tensor.transpose(tp[:, k * 128:(k + 1) * 128],
                                        lns3[k][:, ct * 128:(ct + 1) * 128],
                                        ident)
                nc.scalar.activation(ln3T[:, ct, :], tp.bitcast(F32),
                                      AF.Identity)
                nc.vector.tensor_tensor(ln3lT[:, ct, :], tp.bitcast(F32),
                                        ln3T[:, ct, :], ALU.subtract)
            x3T = mlpp.tile([128, 8, 512], F32, tag="x3T")
            for ct in range(8):
                tp = psmm.tile([128, 512], F32, tag="mm", name="trp3")
                for k in range(4):
                    nc.tensor.transpose(tp[:, k * 128:(k + 1) * 128],
                                        x_sb[:, k, ct * 128:(ct + 1) * 128],
                                        ident32)
                # fold fc2 bias into x3T while evacuating
                nc.scalar.activation(x3T[:, ct, :], tp, AF.Identity,
                                     bias=bfc2T[:, ct:ct + 1])

            h1gT = mlpp.tile([128, 32, 512], FP8, tag="h1gT")
            for og in range(16):
                wch = col_block_dma(w_fc1h, og * 256)
                wcl = col_block_dma(w_fc1l, og * 256)
                for oi in range(2):
                    ot = og * 2 + oi
                    pf = pfp.tile([128, 512], F32, tag="pf", name="pf")
                    passes = [(wch, ln3T), (wcl, ln3T), (wch, ln3lT)]
                    for pi, (wc_, rhs) in enumerate(passes):
                        for kp in range(KT // 2):
                            nc.tensor.matmul(
                                pf,
                                wc_[:, 2 * kp:2 * kp + 2, oi * 128:(oi + 1) * 128],
                                rhs[:, 2 * kp:2 * kp + 2, :],
                                start=(pi == 0 and kp == 0),
                                stop=(pi == 2 and kp == KT // 2 - 1),
                                perf_mode=DR)
                    nc.scalar.activation(h1gT[:, ot, :], pf, AF.Gelu,
                                         bias=bfc1T[:, ot:ot + 1], scale=1.0 / WS)

            for ot in range(8):
                wfl = wfc2p.tile([128, 32, 128], FP8, tag="wfc2", name="wfl")
                nc.gpsimd.dma_start(
                    out=wfl, in_=w_fc2l[:, ot * 128:(ot + 1) * 128].rearrange(
                        "(kt p) o -> p kt o", p=128))
                pm = psmm.tile([128, 512], F32, tag="mm", name="pm")
                for kp in range(16):
                    nc.tensor.matmul(
                        pm, wf2h[:, 2 * kp:2 * kp + 2, ot * 128:(ot + 1) * 128],
                        h1gT[:, 2 * kp:2 * kp + 2, :],
                        start=(kp == 0), stop=False, perf_mode=DR)
                for kp in range(16):
                    nc.tensor.matmul(pm, wfl[:, 2 * kp:2 * kp + 2, :],
                                     h1gT[:, 2 * kp:2 * kp + 2, :],
                                     start=False, stop=(kp == 15),
                                     perf_mode=DR)
                fin = finp.tile([128, 512], F32, tag="fin", name="fin")
                nc.scalar.activation(fin, pm, AF.Identity, scale=1.0 / WS)
                nc.vector.tensor_tensor(fin, fin, x3T[:, ot, :], ALU.add)
                nc.sync.dma_start(out=yT[ot * 128:(ot + 1) * 128, :], in_=fin)

    nc.finalize()
    return nc, dbg


# ---------------- host-side prep ----------------

def _prep_inputs(i, x, c, mask, sa_qkv_w, sa_qkv_b, sa_proj_w, sa_proj_b,
                 ca_q_w, ca_q_b, ca_k_w, ca_k_b, ca_v_w, ca_v_b,
                 ca_proj_w, ca_proj_b, fc1_w, fc1_b, fc2_w, fc2_b):
    import ml_dtypes
    f8 = ml_dtypes.float8_e4m3
    f = np.float32
    b_i, rh = i // 2, i % 2
    r0 = rh * R

    def colT(vec, n):  # bias vector -> [128, n] transposed col layout
        return np.ascontiguousarray(np.asarray(vec, f).reshape(n, 128).T)

    # pack keys: indices of unmasked rows of this batch
    mb = np.asarray(mask[b_i])
    kv_idx = np.nonzero(mb == 1)[0]
    assert len(kv_idx) <= NKV, f"unmasked count {len(kv_idx)} > NKV={NKV}"
    nkv = len(kv_idx)
    x_kv = np.zeros((NKV, C), ml_dtypes.bfloat16)
    x_kv[:nkv] = np.asarray(x[b_i], f)[kv_idx].astype(ml_dtypes.bfloat16)
    c_kv = np.zeros((NKV, C), f)
    c_kv[:nkv] = np.asarray(c[b_i], f)[kv_idx]
    mrow = np.full(NKV, -10000.0, f)
    mrow[:nkv] = 0.0
    mcol = np.ascontiguousarray(mrow.reshape(KYV, 128).T)

    w1s = np.asarray(fc1_w, f) * WS
    w1h = w1s.astype(f8)
    w2s = np.asarray(fc2_w, f) * WS
    w2h = w2s.astype(f8)
    wq = np.asarray(sa_qkv_w[:, 0:C], f) * WS
    wk = np.asarray(sa_qkv_w[:, C:2 * C], f) * WS
    wv = np.asarray(sa_qkv_w[:, 2 * C:3 * C], f)
    w_qkv = np.concatenate([wq, wk, wv], axis=1).astype(f8)
    bq = np.asarray(sa_qkv_b[0:C], f) * WS
    bk = np.asarray(sa_qkv_b[C:2 * C], f) * WS

    return {
        "x_own": np.ascontiguousarray(np.asarray(x[b_i], f)[r0:r0 + R]),
        "x_kv": x_kv,
        "cT": np.ascontiguousarray(c_kv.T).astype(f8),
        "mask_sa": mcol,
        "mask_ca": mcol,
        "w_qkv": w_qkv,
        "w_proj": np.asarray(sa_proj_w, f).astype(f8),
        "w_caq": (np.asarray(ca_q_w, f) * WS).astype(f8),
        "w_cak": (np.asarray(ca_k_w, f) * WS).astype(f8),
        "w_cav": np.asarray(ca_v_w, f).astype(f8),
        "w_caproj": np.asarray(ca_proj_w, f).astype(f8),
        "w_fc1h": w1h,
        "w_fc1l": (w1s - w1h.astype(f)).astype(f8),
        "w_fc2h": w2h,
        "w_fc2l": (w2s - w2h.astype(f)).astype(f8),
        "b_qT": colT(bq, 8),
        "b_kT": colT(bk, 8),
        "b_caqT": colT(np.asarray(ca_q_b, f) * WS, 8),
        "b_cakT": colT(np.asarray(ca_k_b, f) * WS, 8),
        "b_fc1T": colT(np.asarray(fc1_b, f) * WS, 32),
        "b_fc2T": colT(fc2_b, 8),
        "b_vr": np.asarray(sa_qkv_b[2 * C:3 * C], f).reshape(1, C),
        "b_cavr": np.asarray(ca_v_b, f).reshape(1, C),
        "b_projr": np.asarray(sa_proj_b, f).reshape(1, C),
        "b_caprojr": np.asarray(ca_proj_b, f).reshape(1, C),
    }


def kernel(**inputs):
    inputs = {k: np.asarray(v) for k, v in inputs.items()}
    if "prog" not in _cache:
        _cache["prog"] = build_program()[0]
    nc = _cache["prog"]
    in_maps = [_prep_inputs(i, **inputs) for i in range(8)]
    res = run_bass_kernel_spmd(nc, in_maps, core_ids=list(range(8)))
    out = np.empty((B, N, C), np.float32)
    for i in range(8):
        b_i, rh = i // 2, i % 2
        out[b_i, rh * R:(rh + 1) * R, :] = res.results[i]["yT"].T
    return out
